# revision 16
# baseline (speedup 1.0000x reference)
"""GCN message-passing kernel for 8 Trainium2 NeuronCores — v2 (CH=8).

Strategy: shard NODES (destinations) across the 8 cores; each core owns all
edges whose dst lands in its node slice, so no cross-core reduction is needed.
The per-edge feature gather uses batched dma_gather instructions (CH*128 rows
per instruction) instead of one indirect DMA per 128 edges — the v1 baseline
was bottlenecked (77% busy) on per-instruction SWDGE descriptor generation.
The SWDGE descriptor ring limits a single gather to ~128 descriptors per
engine lane, capping CH at 15 (8*CH+1 <= 128); CH=8 is the validated setting.

dma_gather indices are int16, so the feature table is split at row 32768 into
LO/HI halves; each window's edges are grouped into LO tiles then HI tiles
(padded to the 128-edge tile boundary, shared schedule across cores).

Segment-sum per 128-node window via weighted one-hot matmul on the PE
(PSUM f32 accumulate); mean + deg==0 passthrough folded into per-edge weights
(w_e = 1/deg[dst], plus a self-edge with w=1 for isolated nodes). Output is
written transposed [D, NPC] and fixed on host.
"""
import os
import sys
sys.path.insert(0, "/opt/trn_rl_repo")
os.environ.setdefault("NEURON_RT_RESET_CORES", "1")

import numpy as np
import ml_dtypes
import concourse.bass as bass
import concourse.bacc as bacc
import concourse.mybir as mybir
import concourse.tile as tile
from concourse.bass_utils import run_bass_kernel_spmd

P = 128
N_NODES = 50000
N_EDGES = 600000
D = 128
N_CORES = 8
WIN = 128                      # nodes per window (= PSUM tile free size)
WINS_PER_CORE = 49             # 49 * 128 = 6272 node slots per core
NPC = WINS_PER_CORE * WIN      # 6272 node slots (core 7 has ghost tail)
SPLIT = 32768                  # feature table split (int16 gather indices)
CH = 8                         # gather chunk size, in 128-edge tiles

DT = "bf16"                    # message/one-hot dtype: "f32" or "bf16"


def _np_dt():
    return np.float32 if DT == "f32" else ml_dtypes.bfloat16


def _bir_dt():
    return mybir.dt.float32 if DT == "f32" else mybir.dt.bfloat16


def _wrap_idx(arr):
    """int16 stream [L*128] -> dma_gather layout [128, L*8] (16-wrapped,
    replicated to all 128 partitions)."""
    w16 = arr.reshape(-1, 16).T            # [16, L*8]
    return np.ascontiguousarray(np.tile(w16, (8, 1)))


def _host_schedule(feature, W, b, src, dst):
    deg = np.bincount(dst, minlength=N_NODES).astype(np.int64)
    recip = 1.0 / np.maximum(deg, 1).astype(np.float32)

    iso = np.where(deg == 0)[0].astype(np.int64)
    if iso.size:
        src = np.concatenate([src, iso])
        dst = np.concatenate([dst, iso])
    E = src.shape[0]
    w_edge = recip[dst]                    # iso self-edges get w=1

    core = dst // NPC
    dloc = dst - core * NPC
    win = dloc >> 7
    nloc = (dloc & 127).astype(np.float32)
    half = (src >= SPLIT).astype(np.int64)

    # tiles per (win, half): shared across cores (SPMD single program)
    cnt = np.zeros((N_CORES, WINS_PER_CORE, 2), dtype=np.int64)
    np.add.at(cnt, (core, win, half), 1)
    t_grp = -(-cnt.max(axis=0) // P)       # [49, 2] ceil
    t_lo_w, t_hi_w = t_grp[:, 0], t_grp[:, 1]
    T_lo, T_hi = int(t_lo_w.sum()), int(t_hi_w.sum())
    NT = T_lo + T_hi

    # stream tile base per window (within each half's stream)
    lo_tbase = np.concatenate([[0], np.cumsum(t_lo_w)])
    hi_tbase = np.concatenate([[0], np.cumsum(t_hi_w)])
    # global tile index base per window (lo tiles then hi tiles)
    g_tbase = np.concatenate([[0], np.cumsum(t_lo_w + t_hi_w)])
    lo2g = np.zeros(max(T_lo, 1), dtype=np.int64)
    hi2g = np.zeros(max(T_hi, 1), dtype=np.int64)
    for w in range(WINS_PER_CORE):
        lo2g[lo_tbase[w]:lo_tbase[w + 1]] = g_tbase[w] + np.arange(t_lo_w[w])
        hi2g[hi_tbase[w]:hi_tbase[w + 1]] = (g_tbase[w] + t_lo_w[w]
                                             + np.arange(t_hi_w[w]))

    # per-edge slot within its half's stream
    grp = (core * WINS_PER_CORE + win) * 2 + half      # sort group
    order = np.lexsort((grp,))
    s_src, s_core, s_win, s_half = src[order], core[order], win[order], half[order]
    s_nloc, s_w = nloc[order], w_edge[order]
    gcnt = np.bincount(grp, minlength=N_CORES * WINS_PER_CORE * 2)
    gstart = np.concatenate([[0], np.cumsum(gcnt)])
    pos = np.arange(E) - gstart[grp[order]]
    tbase = np.where(s_half == 0, lo_tbase[s_win], hi_tbase[s_win])
    slot = tbase * P + pos                 # slot within the half's stream

    np_dt = _np_dt()
    idx_lo = np.zeros((N_CORES, max(T_lo, 1) * P), dtype=np.int16)
    idx_hi = np.zeros((N_CORES, max(T_hi, 1) * P), dtype=np.int16)
    dstv = np.full((N_CORES, NT * P), -1.0, dtype=np.float32)
    wv = np.zeros((N_CORES, NT * P), dtype=np.float32)

    is_lo = s_half == 0
    idx_lo[s_core[is_lo], slot[is_lo]] = s_src[is_lo].astype(np.int16)
    idx_hi[s_core[~is_lo], slot[~is_lo]] = (s_src[~is_lo] - SPLIT).astype(np.int16)
    # global slot = global_tile*128 + (slot % 128)
    g_tile = np.empty(E, dtype=np.int64)
    g_tile[is_lo] = lo2g[slot[is_lo] >> 7]
    g_tile[~is_lo] = hi2g[slot[~is_lo] >> 7]
    g_slot = g_tile * P + (slot & 127)
    dstv[s_core, g_slot] = s_nloc
    wv[s_core, g_slot] = s_w

    feature = np.ascontiguousarray(feature, dtype=np.float32)
    feat_lo = np.ascontiguousarray(feature[:SPLIT].astype(np_dt))
    feat_hi = np.ascontiguousarray(feature[SPLIT:].astype(np_dt))
    Wf = np.ascontiguousarray(W, dtype=np.float32)
    bf = np.ascontiguousarray(b, dtype=np.float32).reshape(P, 1)

    in_maps = []
    for c in range(N_CORES):
        in_maps.append({
            "feat_lo": feat_lo,
            "feat_hi": feat_hi,
            "idx_lo": _wrap_idx(idx_lo[c]),
            "idx_hi": _wrap_idx(idx_hi[c]),
            "dstv": np.ascontiguousarray(dstv[c].reshape(NT, P).T),
            "wv": np.ascontiguousarray(wv[c].reshape(NT, P).T),
            "Wmat": Wf,
            "bias": bf,
        })
    sched = (tuple(t_lo_w.tolist()), tuple(t_hi_w.tolist()))
    return in_maps, sched


def _build(sched, reps=1, scratch=65536):
    t_lo_w, t_hi_w = (np.asarray(s, dtype=np.int64) for s in sched)
    T_lo, T_hi = int(t_lo_w.sum()), int(t_hi_w.sum())
    NT = T_lo + T_hi
    bdt = _bir_dt()

    nc = bacc.Bacc("TRN2", debug=False, num_devices=N_CORES,
                   dynamic_dma_scratch_size=scratch)
    feat_lo = nc.dram_tensor("feat_lo", [SPLIT, D], bdt, kind="ExternalInput")
    feat_hi = nc.dram_tensor("feat_hi", [N_NODES - SPLIT, D], bdt,
                             kind="ExternalInput")
    idx_lo = nc.dram_tensor("idx_lo", [P, max(T_lo, 1) * 8], mybir.dt.int16,
                            kind="ExternalInput")
    idx_hi = nc.dram_tensor("idx_hi", [P, max(T_hi, 1) * 8], mybir.dt.int16,
                            kind="ExternalInput")
    dstv = nc.dram_tensor("dstv", [P, NT], mybir.dt.float32, kind="ExternalInput")
    wv = nc.dram_tensor("wv", [P, NT], mybir.dt.float32, kind="ExternalInput")
    Wmat = nc.dram_tensor("Wmat", [D, D], mybir.dt.float32, kind="ExternalInput")
    bias = nc.dram_tensor("bias", [P, 1], mybir.dt.float32, kind="ExternalInput")
    out = nc.dram_tensor("out", [D, NPC], mybir.dt.float32, kind="ExternalOutput")

    with tile.TileContext(nc) as tc:
        with (
            tc.tile_pool(name="const", bufs=1) as cpool,
            tc.tile_pool(name="glo", bufs=4) as lopool,
            tc.tile_pool(name="ghi", bufs=4) as hipool,
            tc.tile_pool(name="oh", bufs=8) as ohpool,
            tc.tile_pool(name="hwin", bufs=3) as hpool,
            tc.tile_pool(name="outw", bufs=3) as opool,
            tc.tile_pool(name="ph1", bufs=2, space="PSUM") as p1pool,
            tc.tile_pool(name="ph2", bufs=2, space="PSUM") as p2pool,
        ):
            ilo_t = cpool.tile([P, max(T_lo, 1) * 8], mybir.dt.int16)
            nc.sync.dma_start(ilo_t[:], idx_lo[:])
            ihi_t = cpool.tile([P, max(T_hi, 1) * 8], mybir.dt.int16)
            nc.sync.dma_start(ihi_t[:], idx_hi[:])
            dst_t = cpool.tile([P, NT], mybir.dt.float32)
            nc.sync.dma_start(dst_t[:], dstv[:])
            w_t = cpool.tile([P, NT], mybir.dt.float32)
            nc.sync.dma_start(w_t[:], wv[:])
            Wt = cpool.tile([D, D], mybir.dt.float32)
            nc.sync.dma_start(Wt[:], Wmat[:])
            b_t = cpool.tile([P, 1], mybir.dt.float32)
            nc.sync.dma_start(b_t[:], bias[:])

            iota_i = cpool.tile([P, WIN], mybir.dt.int32)
            nc.gpsimd.iota(iota_i[:], pattern=[[1, WIN]], base=0,
                           channel_multiplier=0)
            iota_f = cpool.tile([P, WIN], bdt)
            nc.vector.tensor_copy(iota_f[:], iota_i[:])

            for rep in range(reps):
                streams = {
                    "lo": {"pool": lopool, "feat": feat_lo, "idx": ilo_t,
                           "T": T_lo, "tiles": {}},
                    "hi": {"pool": hipool, "feat": feat_hi, "idx": ihi_t,
                           "T": T_hi, "tiles": {}},
                }

                def fetch(s, st):
                    """Return the SBUF AP for stream tile st, gathering its
                    chunk if not yet issued."""
                    k = st // CH
                    if k not in s["tiles"]:
                        ntiles = min(CH, s["T"] - k * CH)
                        t = s["pool"].tile([P, CH * D], bdt)
                        ap3 = t[:, :ntiles * D].rearrange(
                            "p (t d) -> p t d", d=D)
                        nc.gpsimd.dma_gather(
                            ap3, s["feat"][:],
                            s["idx"][:, k * CH * 8:(k * CH + ntiles) * 8],
                            ntiles * P, ntiles * P, D)
                        s["tiles"][k] = t
                    j = st % CH
                    return s["tiles"][k][:, j * D:(j + 1) * D]

                gt = 0
                lo_cur = 0
                hi_cur = 0
                for w in range(WINS_PER_CORE):
                    tl, th = int(t_lo_w[w]), int(t_hi_w[w])
                    tw = tl + th
                    ph1 = p1pool.tile([D, WIN], mybir.dt.float32, space="PSUM")
                    k = 0
                    for sname, n_t in (("lo", tl), ("hi", th)):
                        s = streams[sname]
                        for _ in range(n_t):
                            m = fetch(s, lo_cur if sname == "lo" else hi_cur)
                            if sname == "lo":
                                lo_cur += 1
                            else:
                                hi_cur += 1
                            oh = ohpool.tile([P, WIN], bdt)
                            nc.vector.tensor_scalar(
                                out=oh[:], in0=iota_f[:],
                                scalar1=dst_t[:, gt:gt + 1],
                                scalar2=w_t[:, gt:gt + 1],
                                op0=mybir.AluOpType.is_equal,
                                op1=mybir.AluOpType.mult,
                            )
                            # ph1[f, n] += sum_e m[e, f] * oh[e, n]
                            nc.tensor.matmul(out=ph1[:], lhsT=m, rhs=oh[:],
                                             start=(k == 0), stop=(k == tw - 1))
                            gt += 1
                            k += 1

                    # hT window -> SBUF (ACT engine, keeps DVE free)
                    hT = hpool.tile([D, WIN], mybir.dt.float32)
                    nc.scalar.activation(hT[:], ph1[:],
                                         mybir.ActivationFunctionType.Copy,
                                         scale=1.0)
                    # outT = W^T @ hT   [dout, n]
                    ph2 = p2pool.tile([D, WIN], mybir.dt.float32, space="PSUM")
                    nc.tensor.matmul(out=ph2[:], lhsT=Wt[:], rhs=hT[:],
                                     start=True, stop=True)
                    # relu(x + b) -> SBUF, store transposed [dout, n]
                    s2 = opool.tile([D, WIN], mybir.dt.float32)
                    nc.scalar.activation(s2[:], ph2[:],
                                         mybir.ActivationFunctionType.Relu,
                                         bias=b_t[:, 0:1], scale=1.0)
                    nc.sync.dma_start(out[:, w * WIN:(w + 1) * WIN], s2[:])
    nc.compile()
    return nc


_CACHE = {}


def prepared(inputs):
    feature = np.asarray(inputs["feature"], dtype=np.float32)
    W = np.asarray(inputs["W"], dtype=np.float32)
    b = np.asarray(inputs["b"], dtype=np.float32)
    src = np.asarray(inputs["src"], np.int64)
    dst = np.asarray(inputs["dst"], np.int64)
    in_maps, sched = _host_schedule(feature, W, b, src, dst)
    if sched not in _CACHE:
        _CACHE[sched] = _build(sched)
    return _CACHE[sched], in_maps


def kernel(feature, W, b, src, dst):
    inputs = {"feature": feature, "W": W, "b": b, "src": src, "dst": dst}
    nc, in_maps = prepared(inputs)
    res = run_bass_kernel_spmd(nc, in_maps, core_ids=list(range(N_CORES)))
    out = np.empty((N_NODES, D), dtype=np.float32)
    for c in range(N_CORES):
        lo = c * NPC
        hi = min(lo + NPC, N_NODES)
        out[lo:hi] = res.results[c]["out"].T[: hi - lo]
    return out


# revision 17
# speedup vs baseline: 2.2175x; 2.2175x over previous
"""GCN message-passing kernel for 8 Trainium2 NeuronCores — v2 (CH=8).

Strategy: shard NODES (destinations) across the 8 cores; each core owns all
edges whose dst lands in its node slice, so no cross-core reduction is needed.
The per-edge feature gather uses batched dma_gather instructions (CH*128 rows
per instruction) instead of one indirect DMA per 128 edges — the v1 baseline
was bottlenecked (77% busy) on per-instruction SWDGE descriptor generation.
The SWDGE descriptor ring limits a single gather to ~128 descriptors per
engine lane, capping CH at 15 (8*CH+1 <= 128); CH=8 is the validated setting.

dma_gather indices are int16, so the feature table is split at row 32768 into
LO/HI halves; each window's edges are grouped into LO tiles then HI tiles
(padded to the 128-edge tile boundary, shared schedule across cores).

Segment-sum per 128-node window via weighted one-hot matmul on the PE
(PSUM f32 accumulate); mean + deg==0 passthrough folded into per-edge weights
(w_e = 1/deg[dst], plus a self-edge with w=1 for isolated nodes). Output is
written transposed [D, NPC] and fixed on host.
"""
import os
import sys
sys.path.insert(0, "/opt/trn_rl_repo")
os.environ.setdefault("NEURON_RT_RESET_CORES", "1")

import numpy as np
import ml_dtypes
import concourse.bass as bass
import concourse.bacc as bacc
import concourse.mybir as mybir
import concourse.tile as tile
from concourse.bass_utils import run_bass_kernel_spmd

P = 128
N_NODES = 50000
N_EDGES = 600000
D = 128
N_CORES = 8
WIN = 128                      # nodes per window (= PSUM tile free size)
WINS_PER_CORE = 49             # 49 * 128 = 6272 node slots per core
NPC = WINS_PER_CORE * WIN      # 6272 node slots (core 7 has ghost tail)
SPLIT = 32768                  # feature table split (int16 gather indices)
CH = 8                         # gather chunk size, in 128-edge tiles
KOH = 16                       # tiles per batched one-hot build
QUEUES = 4                     # SWDGE queues (Q7 cpu pairs)

DT = "bf16"                    # message/one-hot dtype: "f32" or "bf16"


def _np_dt():
    return np.float32 if DT == "f32" else ml_dtypes.bfloat16


def _bir_dt():
    return mybir.dt.float32 if DT == "f32" else mybir.dt.bfloat16


def _wrap_idx(arr):
    """int16 stream [L*128] -> dma_gather layout [128, L*8] (16-wrapped,
    replicated to all 128 partitions)."""
    w16 = arr.reshape(-1, 16).T            # [16, L*8]
    return np.ascontiguousarray(np.tile(w16, (8, 1)))


def _host_schedule(feature, W, b, src, dst):
    deg = np.bincount(dst, minlength=N_NODES).astype(np.int64)
    recip = 1.0 / np.maximum(deg, 1).astype(np.float32)

    iso = np.where(deg == 0)[0].astype(np.int64)
    if iso.size:
        src = np.concatenate([src, iso])
        dst = np.concatenate([dst, iso])
    E = src.shape[0]
    w_edge = recip[dst]                    # iso self-edges get w=1

    core = dst // NPC
    dloc = dst - core * NPC
    win = dloc >> 7
    nloc = (dloc & 127).astype(np.float32)
    half = (src >= SPLIT).astype(np.int64)

    # tiles per (win, half): shared across cores (SPMD single program)
    cnt = np.zeros((N_CORES, WINS_PER_CORE, 2), dtype=np.int64)
    np.add.at(cnt, (core, win, half), 1)
    t_grp = -(-cnt.max(axis=0) // P)       # [49, 2] ceil
    t_lo_w, t_hi_w = t_grp[:, 0], t_grp[:, 1]
    T_lo, T_hi = int(t_lo_w.sum()), int(t_hi_w.sum())
    NT = T_lo + T_hi

    # stream tile base per window (within each half's stream)
    lo_tbase = np.concatenate([[0], np.cumsum(t_lo_w)])
    hi_tbase = np.concatenate([[0], np.cumsum(t_hi_w)])
    # global tile index base per window (lo tiles then hi tiles)
    g_tbase = np.concatenate([[0], np.cumsum(t_lo_w + t_hi_w)])
    lo2g = np.zeros(max(T_lo, 1), dtype=np.int64)
    hi2g = np.zeros(max(T_hi, 1), dtype=np.int64)
    for w in range(WINS_PER_CORE):
        lo2g[lo_tbase[w]:lo_tbase[w + 1]] = g_tbase[w] + np.arange(t_lo_w[w])
        hi2g[hi_tbase[w]:hi_tbase[w + 1]] = (g_tbase[w] + t_lo_w[w]
                                             + np.arange(t_hi_w[w]))

    # per-edge slot within its half's stream
    grp = (core * WINS_PER_CORE + win) * 2 + half      # sort group
    order = np.lexsort((grp,))
    s_src, s_core, s_win, s_half = src[order], core[order], win[order], half[order]
    s_nloc, s_w = nloc[order], w_edge[order]
    gcnt = np.bincount(grp, minlength=N_CORES * WINS_PER_CORE * 2)
    gstart = np.concatenate([[0], np.cumsum(gcnt)])
    pos = np.arange(E) - gstart[grp[order]]
    tbase = np.where(s_half == 0, lo_tbase[s_win], hi_tbase[s_win])
    slot = tbase * P + pos                 # slot within the half's stream

    np_dt = _np_dt()
    idx_lo = np.zeros((N_CORES, max(T_lo, 1) * P), dtype=np.int16)
    idx_hi = np.zeros((N_CORES, max(T_hi, 1) * P), dtype=np.int16)
    dstv = np.full((N_CORES, NT * P), -1.0, dtype=np.float32)
    wv = np.zeros((N_CORES, NT * P), dtype=np.float32)

    is_lo = s_half == 0
    idx_lo[s_core[is_lo], slot[is_lo]] = s_src[is_lo].astype(np.int16)
    idx_hi[s_core[~is_lo], slot[~is_lo]] = (s_src[~is_lo] - SPLIT).astype(np.int16)
    # global slot = global_tile*128 + (slot % 128)
    g_tile = np.empty(E, dtype=np.int64)
    g_tile[is_lo] = lo2g[slot[is_lo] >> 7]
    g_tile[~is_lo] = hi2g[slot[~is_lo] >> 7]
    g_slot = g_tile * P + (slot & 127)
    dstv[s_core, g_slot] = s_nloc
    wv[s_core, g_slot] = s_w

    feature = np.ascontiguousarray(feature, dtype=np.float32)
    feat_lo = np.ascontiguousarray(feature[:SPLIT].astype(np_dt))
    feat_hi = np.ascontiguousarray(feature[SPLIT:].astype(np_dt))
    Wf = np.ascontiguousarray(W, dtype=np.float32)
    bf = np.ascontiguousarray(b, dtype=np.float32).reshape(P, 1)

    recip_pad = np.zeros(N_CORES * NPC, dtype=np.float32)
    recip_pad[:N_NODES] = recip
    recip_rep = np.broadcast_to(
        recip_pad.reshape(N_CORES, 1, NPC), (N_CORES, P, NPC))

    in_maps = []
    for c in range(N_CORES):
        in_maps.append({
            "feat_lo": feat_lo,
            "feat_hi": feat_hi,
            "idx_lo": _wrap_idx(idx_lo[c]),
            "idx_hi": _wrap_idx(idx_hi[c]),
            "dstv": np.ascontiguousarray(
                dstv[c].reshape(NT, P).T.astype(np_dt)),
            "recipv": np.ascontiguousarray(recip_rep[c]),
            "Wmat": Wf,
            "bias": bf,
        })
    sched = (tuple(t_lo_w.tolist()), tuple(t_hi_w.tolist()))
    return in_maps, sched


def _build(sched, reps=1, scratch=65536):
    t_lo_w, t_hi_w = (np.asarray(s, dtype=np.int64) for s in sched)
    T_lo, T_hi = int(t_lo_w.sum()), int(t_hi_w.sum())
    NT = T_lo + T_hi
    bdt = _bir_dt()

    nc = bacc.Bacc("TRN2", debug=False, num_devices=N_CORES,
                   dynamic_dma_scratch_size=scratch,
                   num_swdge_queues=QUEUES)
    feat_lo = nc.dram_tensor("feat_lo", [SPLIT, D], bdt, kind="ExternalInput")
    feat_hi = nc.dram_tensor("feat_hi", [N_NODES - SPLIT, D], bdt,
                             kind="ExternalInput")
    idx_lo = nc.dram_tensor("idx_lo", [P, max(T_lo, 1) * 8], mybir.dt.int16,
                            kind="ExternalInput")
    idx_hi = nc.dram_tensor("idx_hi", [P, max(T_hi, 1) * 8], mybir.dt.int16,
                            kind="ExternalInput")
    dstv = nc.dram_tensor("dstv", [P, NT], bdt, kind="ExternalInput")
    recipv = nc.dram_tensor("recipv", [P, NPC], mybir.dt.float32,
                            kind="ExternalInput")
    Wmat = nc.dram_tensor("Wmat", [D, D], mybir.dt.float32, kind="ExternalInput")
    bias = nc.dram_tensor("bias", [P, 1], mybir.dt.float32, kind="ExternalInput")
    out = nc.dram_tensor("out", [D, NPC], mybir.dt.float32, kind="ExternalOutput")

    with tile.TileContext(nc) as tc:
        with (
            tc.tile_pool(name="const", bufs=1) as cpool,
            tc.tile_pool(name="glo", bufs=4) as lopool,
            tc.tile_pool(name="ghi", bufs=4) as hipool,
            tc.tile_pool(name="oh", bufs=3) as ohpool,
            tc.tile_pool(name="hwin", bufs=3) as hpool,
            tc.tile_pool(name="outw", bufs=3) as opool,
            tc.tile_pool(name="ph1", bufs=2, space="PSUM") as p1pool,
            tc.tile_pool(name="ph2", bufs=2, space="PSUM") as p2pool,
        ):
            ilo_t = cpool.tile([P, max(T_lo, 1) * 8], mybir.dt.int16)
            nc.sync.dma_start(ilo_t[:], idx_lo[:])
            ihi_t = cpool.tile([P, max(T_hi, 1) * 8], mybir.dt.int16)
            nc.sync.dma_start(ihi_t[:], idx_hi[:])
            dst_t = cpool.tile([P, NT], bdt)
            nc.sync.dma_start(dst_t[:], dstv[:])
            rec_t = cpool.tile([P, NPC], mybir.dt.float32)
            nc.sync.dma_start(rec_t[:], recipv[:])
            Wt = cpool.tile([D, D], mybir.dt.float32)
            nc.sync.dma_start(Wt[:], Wmat[:])
            b_t = cpool.tile([P, 1], mybir.dt.float32)
            nc.sync.dma_start(b_t[:], bias[:])

            iota_i = cpool.tile([P, WIN], mybir.dt.int32)
            nc.gpsimd.iota(iota_i[:], pattern=[[1, WIN]], base=0,
                           channel_multiplier=0)
            iota_f = cpool.tile([P, WIN], bdt)
            nc.vector.tensor_copy(iota_f[:], iota_i[:])

            for rep in range(reps):
                streams = {
                    "lo": {"pool": lopool, "feat": feat_lo, "idx": ilo_t,
                           "T": T_lo, "tiles": {}},
                    "hi": {"pool": hipool, "feat": feat_hi, "idx": ihi_t,
                           "T": T_hi, "tiles": {}},
                }

                qctr = [0]

                def fetch(s, st):
                    """Return the SBUF AP for stream tile st, gathering its
                    chunk if not yet issued."""
                    k = st // CH
                    if k not in s["tiles"]:
                        ntiles = min(CH, s["T"] - k * CH)
                        t = s["pool"].tile([P, CH * D], bdt)
                        ap3 = t[:, :ntiles * D].rearrange(
                            "p (t d) -> p t d", d=D)
                        nc.gpsimd.dma_gather(
                            ap3, s["feat"][:],
                            s["idx"][:, k * CH * 8:(k * CH + ntiles) * 8],
                            ntiles * P, ntiles * P, D,
                            queue_num=qctr[0] % QUEUES)
                        qctr[0] += 1
                        s["tiles"][k] = t
                    j = st % CH
                    return s["tiles"][k][:, j * D:(j + 1) * D]

                oh_tiles = {}

                def get_oh(gt):
                    g = gt // KOH
                    if g not in oh_tiles:
                        n_t = min(KOH, NT - g * KOH)
                        t = ohpool.tile([P, KOH * WIN], bdt)
                        in0 = dst_t[:, g * KOH:g * KOH + n_t].rearrange(
                            "p (k o) -> p k o", o=1).broadcast_to([P, n_t, WIN])
                        in1 = iota_f[:].rearrange(
                            "p (o n) -> p o n", o=1).broadcast_to([P, n_t, WIN])
                        out3 = t[:, :n_t * WIN].rearrange(
                            "p (k n) -> p k n", n=WIN)
                        nc.vector.tensor_tensor(
                            out=out3, in0=in0, in1=in1,
                            op=mybir.AluOpType.is_equal)
                        oh_tiles[g] = t
                    j = gt % KOH
                    return oh_tiles[g][:, j * WIN:(j + 1) * WIN]

                gt = 0
                lo_cur = 0
                hi_cur = 0
                for w in range(WINS_PER_CORE):
                    tl, th = int(t_lo_w[w]), int(t_hi_w[w])
                    tw = tl + th
                    ph1 = p1pool.tile([D, WIN], mybir.dt.float32, space="PSUM")
                    k = 0
                    for sname, n_t in (("lo", tl), ("hi", th)):
                        s = streams[sname]
                        for _ in range(n_t):
                            m = fetch(s, lo_cur if sname == "lo" else hi_cur)
                            if sname == "lo":
                                lo_cur += 1
                            else:
                                hi_cur += 1
                            oh = get_oh(gt)
                            # ph1[f, n] += sum_e m[e, f] * oh[e, n]
                            nc.tensor.matmul(out=ph1[:], lhsT=m, rhs=oh,
                                             start=(k == 0), stop=(k == tw - 1))
                            gt += 1
                            k += 1

                    # hT = ph1 * (1/deg) per node column  -> SBUF f32
                    hT = hpool.tile([D, WIN], mybir.dt.float32)
                    nc.vector.tensor_tensor(
                        out=hT[:], in0=ph1[:],
                        in1=rec_t[:, w * WIN:(w + 1) * WIN],
                        op=mybir.AluOpType.mult)
                    # outT = W^T @ hT   [dout, n]
                    ph2 = p2pool.tile([D, WIN], mybir.dt.float32, space="PSUM")
                    nc.tensor.matmul(out=ph2[:], lhsT=Wt[:], rhs=hT[:],
                                     start=True, stop=True)
                    # relu(x + b) -> SBUF, store transposed [dout, n]
                    s2 = opool.tile([D, WIN], mybir.dt.float32)
                    nc.scalar.activation(s2[:], ph2[:],
                                         mybir.ActivationFunctionType.Relu,
                                         bias=b_t[:, 0:1], scale=1.0)
                    nc.sync.dma_start(out[:, w * WIN:(w + 1) * WIN], s2[:])
    nc.compile()
    return nc


_CACHE = {}


def prepared(inputs):
    feature = np.asarray(inputs["feature"], dtype=np.float32)
    W = np.asarray(inputs["W"], dtype=np.float32)
    b = np.asarray(inputs["b"], dtype=np.float32)
    src = np.asarray(inputs["src"], np.int64)
    dst = np.asarray(inputs["dst"], np.int64)
    in_maps, sched = _host_schedule(feature, W, b, src, dst)
    if sched not in _CACHE:
        _CACHE[sched] = _build(sched)
    return _CACHE[sched], in_maps


def kernel(feature, W, b, src, dst):
    inputs = {"feature": feature, "W": W, "b": b, "src": src, "dst": dst}
    nc, in_maps = prepared(inputs)
    res = run_bass_kernel_spmd(nc, in_maps, core_ids=list(range(N_CORES)))
    out = np.empty((N_NODES, D), dtype=np.float32)
    for c in range(N_CORES):
        lo = c * NPC
        hi = min(lo + NPC, N_NODES)
        out[lo:hi] = res.results[c]["out"].T[: hi - lo]
    return out
